# revision 19
# baseline (speedup 1.0000x reference)
"""FPS (farthest point sampling) Trainium2 kernel, v2.

Problem: x (64, 65536, 3) fp32 -> y (64, 2048, 3): per cloud, iteratively
select the point maximizing min-distance-to-selected-set, starting at index 0
(exact argmax semantics incl. first-index tie-breaks).

Sharding: data-parallel over batch. 8 clouds per core; inside a core, 2
groups of 4 clouds processed as [128 partitions x 2048 free] planes
(cloud = 32 partitions), ping-ponged so one group's argmax tail overlaps the
other group's distance passes. Per FPS iteration (all on-chip):
  ACT   : dx2/dy2/dz2 = Square(coord + (-p_coord))        (3 passes)
  Pool  : t = dx2 + dy2                                   (1 pass)
  DVE   : s = t + dz2                                     (1 pass)
  DVE   : md = min(md, s) fused with pm = max(md)         (1 pass, TTR)
  DVE   : idx8 = max_index(pm8, md)                       (1 pass)
  small : enc = BIGK - (partition*2048 + idx); PE transpose [pm|enc];
          per-cloud (is_ge gm)*enc -> winner enc; log enc; row = kcg - enc
  DMA   : indirect-gather winning x rows -> PE broadcast -coords (npc)
Winner encodings are logged in SBUF and written out once at the end; the
final y gather happens on the host (y = x[rows]), so no per-iteration
y-scatter DMAs. Ties are exact: within a partition max_index returns the
first occurrence; across partitions max of enc = smallest global index.
"""
import sys
import types
import numpy as np

B, N, M = 64, 65536, 2048
NCORES = 8
BPC = B // NCORES          # clouds per core = 8
NGROUPS = 2
CPG = BPC // NGROUPS       # clouds per group = 4
PP = 128 // CPG            # partitions per cloud = 32
FD = N // PP               # free dim per partition = 2048
BIGK = float(1 << 24)
FLT_MAX = 3.4028235e38

_cached = {}


def _install_compat():
    """Environment workarounds: NTFF hook shim + 1-sync-wait-per-instruction
    splitter for this walrus build."""
    try:
        from antenv import axon_hooks  # noqa: F401
    except ImportError:
        try:
            from trn_agent_boot.trn_boot import _ntff_profile_via_ctypes
            _hook = _ntff_profile_via_ctypes('/opt/axon/libaxon_pjrt.so')
        except Exception:
            _hook = None
        _mod = types.ModuleType("antenv.axon_hooks")
        _mod.get_axon_ntff_profile_hook = lambda: _hook
        _mod.set_axon_ntff_profile_hook = lambda h: None
        sys.modules['antenv.axon_hooks'] = _mod

    import concourse.tile as tile_mod
    import concourse.mybir as mybir
    from bass_rust import ScopedClock
    import bass_rust as _br

    if getattr(tile_mod.TileContext, "_fps_patched", False):
        return
    tile_mod.TileContext._fps_patched = True

    _orig_lower = tile_mod.TileContext._lower_ordered_insts

    def _split_waits(self, ordered):
        sem_ids = {}
        try:
            for nm, h in self.sems.allocated().items():
                sem_ids[getattr(h, "name", nm)] = h.num
        except Exception:
            pass
        for bb_name, insts in ordered.items():
            out = []
            for inst in insts:
                si = inst.sync_info
                if type(inst).__name__ == "InstIncSwdgeSem":
                    # This walrus can't encode IncSwdgeSem (extended ISA).
                    # Replace with per-sem NOPs: one wait + one sem-inc each
                    # (mode 'sub' -> negative increments).
                    names = inst._sem_names
                    vals = inst._sem_values
                    mode = str(inst._mode)
                    sgn = -1 if "sub" in mode else 1
                    waits = {w.ant_name: w for w in (
                        list(si.on_wait) if si is not None else [])}
                    for nm, v in zip(names, vals):
                        upd = _br.SyncUpdate(
                            sync_type='semaphore', id=sem_ids[nm],
                            ant_name=nm, update_mode='sem-inc',
                            update_value=sgn * v, update_reg=None)
                        w = waits.pop(nm, None)
                        nop = mybir.InstNoOp(
                            name=self.nc.get_next_instruction_name(),
                            engine=inst.engine,
                            sync_info=mybir.SyncInfo(
                                on_wait=[w] if w is not None else [],
                                on_update=[upd]),
                            bass_nofuse=True,
                        )
                        out.append(nop)
                    for w in waits.values():
                        nop = mybir.InstNoOp(
                            name=self.nc.get_next_instruction_name(),
                            engine=inst.engine,
                            sync_info=mybir.SyncInfo(on_wait=[w], on_update=[]),
                            bass_nofuse=True,
                        )
                        out.append(nop)
                    continue
                if si is not None and len(si.on_wait) > 1:
                    waits = list(si.on_wait)
                    for w in waits[:-1]:
                        nop = mybir.InstNoOp(
                            name=self.nc.get_next_instruction_name(),
                            engine=inst.engine,
                            sync_info=mybir.SyncInfo(on_wait=[w], on_update=[]),
                            bass_nofuse=True,
                        )
                        out.append(nop)
                    si.on_wait = waits[-1:]
                    inst.sync_info = si
                out.append(inst)
            insts[:] = out
        return _orig_lower(self, ordered)

    tile_mod.TileContext._lower_ordered_insts = _split_waits

    def _patched_drain_and_barrier(self, tick_clock, wait_clock):
        probe = self.nc.sync.nop(nofuse=True)
        wait_clock.add_sem_waits(
            probe.ins, ScopedClock({None: tick_clock.global_clock})
        )
        si = probe.ins.sync_info
        waits = list(si.on_wait)
        if len(waits) > 1:
            si.on_wait = waits[:1]
            probe.ins.sync_info = si
            for w in waits[1:]:
                extra = self.nc.sync.nop(nofuse=True)
                extra.ins.sync_info = _br.SyncInfo(on_wait=[w], on_update=[])
        self.nc.sync.drain()
        self.nc.all_engine_barrier()
        assert self.sems is not None
        popped = self.nc._tile_sem_poison_stack.pop()
        assert popped is self._sem_poison
        # NOTE: skip gpsimd dma_reset/sem_clear (range sem_clear emits an
        # InstISA this walrus rejects); only do the free-list bookkeeping.
        sems = list(self.sems.allocated().values())
        if sems:
            sem_nums = [getattr(s_, "num", s_) for s_ in sems]
            self.nc._state.prepend_free_semaphores(sem_nums)
            for poison_set in self.nc._tile_sem_poison_stack:
                poison_set.update(sem_nums)
        self.nc.all_engine_barrier()

    tile_mod.TileContext._drain_and_barrier = _patched_drain_and_barrier


def _build(n_iters=M):
    import concourse.bass as bass
    import concourse.mybir as mybir
    from concourse.tile import TileContext
    from concourse.bass import IndirectOffsetOnAxis

    fp = mybir.dt.float32
    nc = bass.Bass("TRN2", target_bir_lowering=False, debug=False)

    x_d = nc.dram_tensor("x", [BPC * N, 3], fp, kind="ExternalInput")
    rows_d = nc.dram_tensor("rows_out", [NGROUPS * CPG, M], mybir.dt.int32,
                            kind="ExternalOutput")
    # host-precomputed constants (identity, membership, partition bases)
    ident_d = nc.dram_tensor("ident", [128, 128], fp, kind="ExternalInput")
    negmemb_d = nc.dram_tensor("negmemb", [CPG, 128], fp, kind="ExternalInput")
    pbase_d = nc.dram_tensor("pbase", [128, 1], fp, kind="ExternalInput")
    kcg_d = nc.dram_tensor("kcg", [1, NGROUPS * CPG], fp, kind="ExternalInput")
    rows0_d = nc.dram_tensor("rows0", [NGROUPS * CPG, 1], mybir.dt.int32,
                             kind="ExternalInput")

    with TileContext(nc) as tc:
        import contextlib
        with contextlib.ExitStack() as ctx:
            cpool = ctx.enter_context(tc.tile_pool(name="consts", bufs=1))
            ident = cpool.tile([128, 128], fp, tag="ident")
            nc.sync.dma_start(ident[:, :], ident_d[:, :])
            negmemb = cpool.tile([CPG, 128], fp, tag="negmemb")
            nc.sync.dma_start(negmemb[:, :], negmemb_d[:, :])
            pbase = cpool.tile([128, 1], fp, tag="pbase")
            nc.sync.dma_start(pbase[:, :], pbase_d[:, :])
            kcg = cpool.tile([1, NGROUPS * CPG], fp, tag="kcg")
            nc.sync.dma_start(kcg[:, :], kcg_d[:, :])
            ones8 = cpool.tile([128, 8], fp, tag="ones8")
            nc.vector.memset(ones8[:, :], 1.0)

            G = []  # per-group state
            for g in range(NGROUPS):
                gp = ctx.enter_context(tc.tile_pool(name=f"g{g}", bufs=1))
                pg = ctx.enter_context(
                    tc.tile_pool(name=f"p{g}", bufs=1, space="PSUM"))
                st = {}
                for nm in ("xs", "ys", "zs", "md", "dx2", "dy2", "dz2"):
                    st[nm] = gp.tile([128, FD], fp, tag=nm, name=f"{nm}_{g}")
                st["pm8"] = gp.tile([128, 8], fp, tag="pm8", name=f"pm8_{g}")
                st["idx8"] = gp.tile([128, 8], mybir.dt.uint32, tag="idx8",
                                     name=f"idx8_{g}")
                st["pe2"] = gp.tile([128, 2], fp, tag="pe2", name=f"pe2_{g}")
                st["gm4"] = gp.tile([1, CPG], fp, tag="gm4", name=f"gm4_{g}")
                st["wB"] = gp.tile([1, 128], fp, tag="wB", name=f"wB_{g}")
                st["pmT"] = gp.tile([1, 128], fp, tag="pmT", name=f"pmT_{g}")
                st["wenc"] = gp.tile([1, CPG], fp, tag="wenc",
                                     name=f"wenc_{g}")
                st["rowf"] = gp.tile([1, CPG], fp, tag="rowf", name=f"rowf_{g}")
                st["rlog"] = gp.tile([CPG, M], mybir.dt.int32, tag="rlog",
                                     name=f"rlog_{g}")
                st["pts"] = gp.tile([CPG, 3], fp, tag="pts", name=f"pts_{g}")
                st["npc"] = gp.tile([128, 3], fp, tag="npc", name=f"npc_{g}")
                st["ps_t"] = pg.tile([1, 256], fp, tag=f"ps_t{g}",
                                     name=f"ps_t_{g}")
                st["ps_rows"] = pg.tile([CPG, 1], fp, tag=f"ps_rows{g}",
                                        name=f"ps_rows_{g}")
                st["ps_c"] = pg.tile([128, 3], fp, tag=f"ps_c{g}",
                                     name=f"ps_c_{g}")
                G.append(st)

                nc.vector.memset(st["rlog"][:, :], 0)

                # load x contiguously, then split into coordinate planes
                xall = gp.tile([128, FD * 3], fp, tag="xall",
                               name=f"xall_{g}")
                xv2 = x_d.ap().rearrange("(p f) c -> p (f c)", f=FD)
                base = g * CPG * PP
                for sl in range(0, 128, 16):
                    nc.sync.dma_start(
                        xall[sl:sl + 16, :],
                        xv2[base + sl:base + sl + 16, :])
                x3 = xall[:, :].rearrange("p (f c) -> p f c", c=3)
                for nm, c in (("xs", 0), ("ys", 1), ("zs", 2)):
                    nc.vector.tensor_copy(st[nm][:, :], x3[:, :, c])
                nc.vector.memset(st["md"][:, :], FLT_MAX)

                # initial point = index 0 of each cloud
                nc.sync.dma_start(
                    st["rlog"][:, 0:1], rows0_d[g * CPG:(g + 1) * CPG, :])
                nc.gpsimd.indirect_dma_start(
                    out=st["pts"][:, :], out_offset=None,
                    in_=x_d[:, :],
                    in_offset=IndirectOffsetOnAxis(
                        ap=st["rlog"][:, 0:1], axis=0),
                )
                # ps_c = -coords broadcast per partition
                nc.tensor.matmul(
                    st["ps_c"][:, :], negmemb[:, :], st["pts"][:, :])
                nc.scalar.copy(st["npc"][:, :], st["ps_c"][:, :])

            from concourse.tile import add_dep_helper
            last_tail = {}

            def emit_iter(t):
                for g in range(NGROUPS):
                    st = G[g]
                    npc = st["npc"]
                    # squares (ACT)
                    nc.scalar.activation(
                        st["dx2"][:, :], st["xs"][:, :],
                        mybir.ActivationFunctionType.Square,
                        bias=npc[:, 0:1], scale=1.0)
                    nc.scalar.activation(
                        st["dy2"][:, :], st["ys"][:, :],
                        mybir.ActivationFunctionType.Square,
                        bias=npc[:, 1:2], scale=1.0)
                    nc.scalar.activation(
                        st["dz2"][:, :], st["zs"][:, :],
                        mybir.ActivationFunctionType.Square,
                        bias=npc[:, 2:3], scale=1.0)
                    # t = dx2 + dy2, into dy2
                    tadd = nc.vector.tensor_tensor(
                        out=st["dy2"][:, :], in0=st["dx2"][:, :],
                        in1=st["dy2"][:, :], op=mybir.AluOpType.add)
                    other_tail = last_tail.get(1 - g)
                    if other_tail is not None:
                        add_dep_helper(tadd.ins, other_tail.ins, sync=False,
                                       reason="serialize group tails on DVE")
                    # s = t + dz2, into dx2
                    nc.vector.tensor_tensor(
                        out=st["dx2"][:, :], in0=st["dy2"][:, :],
                        in1=st["dz2"][:, :], op=mybir.AluOpType.add)
                    # md = min(md, s)
                    nc.vector.tensor_tensor(
                        out=st["md"][:, :], in0=st["md"][:, :],
                        in1=st["dx2"][:, :], op=mybir.AluOpType.min)
                    # per-partition top-8 + first-index (DVE)
                    nc.vector.max(out=st["pm8"][:, :], in_=st["md"][:, :])
                    nc.vector.max_index(
                        out=st["idx8"][:, :], in_max=st["pm8"][:, :],
                        in_values=st["md"][:, :])
                    # enc = pbase - idx = BIGK - (p*FD + f)  (Pool tiny)
                    nc.gpsimd.tensor_scalar(
                        out=st["pe2"][:, 1:2], in0=st["idx8"][:, 0:1],
                        scalar1=-1.0, scalar2=pbase[:, 0:1],
                        op0=mybir.AluOpType.mult, op1=mybir.AluOpType.add)
                    # transpose pm, enc -> [1, 128] halves of one PSUM row
                    ps_tA = st["ps_t"][0:1, 0:128]
                    ps_tB = st["ps_t"][0:1, 128:256]
                    nc.tensor.transpose(
                        ps_tA, st["pm8"][:, 0:1], ident[:, :])
                    nc.tensor.transpose(
                        ps_tB, st["pe2"][:, 1:2], ident[:, :])
                    # copy pm row to SBUF (stt allows only one PSUM input)
                    nc.vector.tensor_copy(st["pmT"][0:1, :], ps_tA)
                    # per-cloud max pm
                    pmv = st["pmT"][0:1, :].rearrange("o (c p) -> o c p", p=PP)
                    nc.vector.reduce_max(
                        out=st["gm4"][:, :], in_=pmv, axis=mybir.AxisListType.X)
                    # per cloud: winner enc = max((pm' >= gm_c) * enc')
                    for c in range(CPG):
                        nc.vector.scalar_tensor_tensor(
                            out=st["wB"][0:1, c * PP:(c + 1) * PP],
                            in0=st["pmT"][0:1, c * PP:(c + 1) * PP],
                            scalar=st["gm4"][0:1, c:c + 1],
                            in1=ps_tB[0:1, c * PP:(c + 1) * PP],
                            op0=mybir.AluOpType.is_ge,
                            op1=mybir.AluOpType.mult)
                    wv = st["wB"][0:1, :].rearrange("o (c p) -> o c p", p=PP)
                    nc.vector.reduce_max(
                        out=st["wenc"][:, :], in_=wv, axis=mybir.AxisListType.X)
                    # rows = K_g - enc; to [CPG,1] partitions; cast int32
                    nc.gpsimd.tensor_scalar(
                        out=st["rowf"][:, :], in0=st["wenc"][:, :],
                        scalar1=-1.0, scalar2=BIGK + g * CPG * N,
                        op0=mybir.AluOpType.mult, op1=mybir.AluOpType.add)
                    nc.tensor.matmul(
                        st["ps_rows"][:, :], st["rowf"][:, :],
                        ident[0:1, 0:1])
                    rowslot = st["rlog"][:, t:t + 1]
                    rows_copy = nc.vector.tensor_copy(
                        rowslot, st["ps_rows"][:, :])
                    last_tail[g] = rows_copy
                    # gather winners' coords; broadcast -coords
                    nc.gpsimd.indirect_dma_start(
                        out=st["pts"][:, :], out_offset=None,
                        in_=x_d[:, :],
                        in_offset=IndirectOffsetOnAxis(
                            ap=rowslot, axis=0),
                    )
                    nc.tensor.matmul(
                        st["ps_c"][:, :], negmemb[:, :], st["pts"][:, :])
                    nc.scalar.copy(st["npc"][:, :], st["ps_c"][:, :])

            for t in range(1, n_iters):
                emit_iter(t)

            for g in range(NGROUPS):
                nc.sync.dma_start(
                    rows_d[g * CPG:(g + 1) * CPG, :], G[g]["rlog"][:, :])
    return nc


def _host_consts():
    ident = np.eye(128, dtype=np.float32)
    negmemb = np.zeros((CPG, 128), dtype=np.float32)
    for c in range(CPG):
        negmemb[c, c * PP:(c + 1) * PP] = -1.0
    pbase = (BIGK - np.arange(128, dtype=np.float64) * FD).astype(
        np.float32).reshape(128, 1)
    kcg = np.zeros((1, NGROUPS * CPG), dtype=np.float32)
    for g in range(NGROUPS):
        kcg[0, g * CPG:(g + 1) * CPG] = BIGK + g * CPG * N
    rows0 = (np.arange(BPC, dtype=np.int32) * N).reshape(NGROUPS * CPG, 1)
    return ident, negmemb, pbase, kcg, rows0


def run_device(x, n_iters=M, trace=False):
    """Run the device part; returns (enc arrays per core, exec_time_ns)."""
    _install_compat()
    from concourse.bass_utils import run_bass_kernel_spmd

    key = ("nc", n_iters)
    if key not in _cached:
        _cached[key] = _build(n_iters)
    nc = _cached[key]

    ident, negmemb, pbase, kcg, rows0 = _host_consts()
    x = np.ascontiguousarray(x, dtype=np.float32)
    in_maps = []
    for core in range(NCORES):
        shard = x[core * BPC:(core + 1) * BPC].reshape(BPC * N, 3)
        in_maps.append({
            "x": shard, "ident": ident, "negmemb": negmemb,
            "pbase": pbase, "kcg": kcg, "rows0": rows0,
        })
    res = run_bass_kernel_spmd(nc, in_maps, core_ids=list(range(NCORES)),
                               trace=trace)
    _cached["last_res"] = res
    rows = [res.results[i]["rows_out"] for i in range(NCORES)]
    return rows, res.exec_time_ns


def decode_rows(rows_list, n_iters=M):
    """rows arrays (per core [BPC, M] global shard rows) -> (B, n_iters)."""
    idx = np.zeros((B, n_iters), dtype=np.int64)
    for core in range(NCORES):
        rows = rows_list[core].astype(np.int64)[:, :n_iters]  # [BPC, n_iters]
        idx[core * BPC:(core + 1) * BPC] = rows % N
    return idx


def kernel(x: np.ndarray) -> np.ndarray:
    x = np.ascontiguousarray(x, dtype=np.float32)
    rows_list, _ = run_device(x)
    idx = decode_rows(rows_list)
    y = np.take_along_axis(x, idx[:, :, None].astype(np.int64), axis=1)
    return y.astype(np.float32)
